# revision 66
# baseline (speedup 1.0000x reference)
"""Attention-pooling kernel for Trainium2 (8 NeuronCores, SPMD data-parallel).

Computes, for x: [B, S, H] and w: [H, 1]:
    scores[b, s] = sum_h tanh(x[b, s, h]) * w[h]
    attn = softmax(scores, axis=s)
    out[b, h]   = sum_s attn[b, s] * x[b, s, h]

Sharding: data-parallel over batch B across 8 cores (32 batches/core),
w replicated. No inter-core communication; host concatenates the shards.

Per-core dataflow (per batch b), s-tile t in [0, 32), s = p*32 + t:
  DMA   : x[b] -> SBUF as [128 part, 32 tile, 128 h]  (16 KB contiguous
          per partition; float32r-typed view of the same bytes)
  ACT   : energy = tanh(x), split into a GPSIMD range [0, GS) and a DVE
          range [GS, 32) so each multiply has one owner engine
  GPSIMD: eg *= w  (in place);  DVE: ev *= w  (in place)
  DVE   : scores = reduce_add(energy, axis=h)          [128, 32]
  ACT   : ebuf = exp(scores) (float32r), accum_out -> rowsum [128, 1]
  PE    : context via fp32r M=1 matmuls over tile pairs (the fp32r fast
          path needs a moving free size >= 256): even tiles accumulate
          into ps_even[0, 0:128], odd tiles into ps_odd[0, 128:256];
          the unused half of each stream is discarded.  Both useful
          halves land on partition 0, so engines can read them.
  PE    : total = rowsum.T @ ones    [1, 1]
  ACT   : hb = copy(ps_odd[0, 128:256])   (ACT sits close to PSUM)
  DVE   : sum_row = ps_even[0, 0:128] + hb;  recip = 1/total
  ACT   : out_row = sum_row * recip  (scale-AP copy); DMA 512 B -> out[b]

The epilogue of batch b is deferred two batches (emitted after batch
b+2's pair-matmuls) so ACT's in-order stream doesn't stall the front
of later chains on PE: distance 2 measured -18 us vs distance 1.
Softmax normalization is algebraically factored out of the weighted sum
(exp without max-subtraction is safe: |scores| < ~40 here).
"""

import numpy as np

import concourse.bass as bass
import concourse.tile as tile
from concourse import bacc, mybir
from concourse.bass_utils import run_bass_kernel_spmd

B, S, H = 256, 4096, 128
N_CORES = 8
B_SHARD = B // N_CORES  # 32
P = 128                 # SBUF partitions; also H
S_TILES = S // P        # 32  (s = p * S_TILES + t)

F32 = mybir.dt.float32
F32R = mybir.dt.float32r

# s-tiles [0, GS) of the score multiply run on GPSIMD, [GS, S_TILES) on
# DVE. HW-swept: 16 is a sharp optimum (273.8 us; 21 -> 296, 22 -> 356) -
# lighter GPSIMD SBUF traffic decongests DVE/DMA more than the extra DVE
# multiply tiles cost.
GS = 14

_nc_cache = None


def _build() -> bass.Bass:
    nc = bacc.Bacc(None, target_bir_lowering=False, enable_partition_id=False)

    x_ext = nc.declare_dram_parameter(
        "encoder_outputs", [B_SHARD, S, H], F32, isOutput=False
    )
    w_ext = nc.declare_dram_parameter(
        "attention_weights", [H, 1], F32, isOutput=False
    )
    out_ext = nc.declare_dram_parameter("out", [B_SHARD, H], F32, isOutput=True)

    gs = max(1, min(GS, S_TILES - 1))
    vs = S_TILES - gs

    with tile.TileContext(nc) as tc:
        with (
            tc.tile_pool(name="singles", bufs=1) as singles,
            tc.tile_pool(name="xpool", bufs=6) as xpool,
            tc.tile_pool(name="egpool", bufs=4) as egpool,
            tc.tile_pool(name="evpool", bufs=4) as evpool,
            tc.tile_pool(name="small", bufs=8) as small,
            tc.tile_pool(name="psum_ctx", bufs=3, space="PSUM") as psum_ctx_pool,
            tc.tile_pool(name="psum_tot", bufs=2, space="PSUM") as psum_tot_pool,
        ):
            # w broadcast across partitions: w_bcast[p, h] = w[h]
            w_bcast = singles.tile([P, H], F32)
            w_flat = w_ext[:].rearrange("h one -> (one h)")
            w_row = bass.AP(
                tensor=w_flat.tensor,
                offset=w_flat.offset,
                ap=[[0, P], w_flat.ap[0]],
            )
            nc.sync.dma_start(out=w_bcast, in_=w_row)

            ones_col = singles.tile([P, 1], F32)
            nc.vector.memset(ones_col, 1.0)

            # w replicated along the tile axis, one private copy per
            # consumer engine (concurrent same-address reads from two
            # engines contend on SBUF ports)
            w_rep_g = singles.tile([P, gs, H], F32)
            for t in range(gs):
                nc.vector.tensor_copy(w_rep_g[:, t, :], w_bcast)
            w_rep_v = singles.tile([P, vs, H], F32)
            for t in range(vs):
                nc.vector.tensor_copy(w_rep_v[:, t, :], w_bcast)

            # [b, p, t, h] view of DRAM; partition p reads 16 KB contiguous
            xv = x_ext[:].rearrange("b (p t) h -> b p t h", p=P)

            # Per-batch stages; state is carried in dicts because batch
            # b's epilogue (stage5) is emitted two batches late.
            st = [dict() for _ in range(B_SHARD)]

            def stage0(b, d):  # load
                d["xb"] = xb = xpool.tile([P, S_TILES, H], F32R, tag="xb", name="xb")
                nc.sync.dma_start(out=xb, in_=xv[b].bitcast(F32R))

            def stage1(b, d):  # tanh
                xbf = d["xb"].bitcast(F32)
                d["eg"] = eg = egpool.tile([P, gs, H], F32, tag="eg", name="eg")
                d["ev"] = ev = evpool.tile([P, vs, H], F32, tag="ev", name="ev")
                nc.scalar.activation(
                    out=eg,
                    in_=xbf[:, 0:gs, :],
                    func=mybir.ActivationFunctionType.Tanh,
                )
                nc.scalar.activation(
                    out=ev,
                    in_=xbf[:, gs:, :],
                    func=mybir.ActivationFunctionType.Tanh,
                )

            def stage2(b, d):  # score multiply (split GPSIMD / DVE),
                # both in place: out-of-place DVE TT measured ~25% slower
                eg, ev = d["eg"], d["ev"]
                nc.vector.tensor_mul(ev, ev, w_rep_v)
                nc.gpsimd.tensor_mul(eg, eg, w_rep_g)

            def stage3(b, d):  # reduce + exp (each engine reduces its
                # own range, so the two score chains stay decoupled)
                scores = small.tile([P, S_TILES], F32, tag="scores")
                nc.vector.tensor_reduce(
                    out=scores[:, gs:],
                    in_=d["ev"],
                    axis=mybir.AxisListType.X,
                    op=mybir.AluOpType.add,
                )
                nc.vector.tensor_reduce(
                    out=scores[:, 0:gs],
                    in_=d["eg"],
                    axis=mybir.AxisListType.X,
                    op=mybir.AluOpType.add,
                )
                d["ebuf"] = ebuf = small.tile([P, S_TILES], F32R, tag="ebuf", name="ebuf")
                d["rowsum"] = rowsum = small.tile([P, 1], F32, tag="rowsum", name="rowsum")
                nc.scalar.activation(
                    out=ebuf,
                    in_=scores,
                    func=mybir.ActivationFunctionType.Exp,
                    accum_out=rowsum,
                )

            def stage4(b, d):  # fp32r pair-matmuls
                xb, ebuf = d["xb"], d["ebuf"]
                ps_even = psum_ctx_pool.tile([1, 2 * H], F32, tag="ps_even")
                ps_odd = psum_ctx_pool.tile([1, 2 * H], F32, tag="ps_odd")
                npairs = S_TILES // 2
                for j in range(npairs):
                    rhs = xb[:, 2 * j : 2 * j + 2, :]
                    nc.tensor.matmul(
                        ps_even,
                        ebuf[:, 2 * j : 2 * j + 1],
                        rhs,
                        start=(j == 0),
                        stop=(j == npairs - 1),
                    )
                    nc.tensor.matmul(
                        ps_odd,
                        ebuf[:, 2 * j + 1 : 2 * j + 2],
                        rhs,
                        start=(j == 0),
                        stop=(j == npairs - 1),
                    )

                tot_ps = psum_tot_pool.tile([1, 1], F32)
                nc.tensor.matmul(
                    tot_ps, d["rowsum"], ones_col, start=True, stop=True
                )
                d["ps_even"], d["ps_odd"], d["tot_ps"] = ps_even, ps_odd, tot_ps

            def stage5(b, d):  # normalize + store (emitted one batch late
                # so ACT's PSUM-gated epilogue ops don't sit between the
                # next batch's tanh ops in ACT's in-order stream)
                ps_even, ps_odd, tot_ps = d["ps_even"], d["ps_odd"], d["tot_ps"]
                # ctx = ps_even[0, 0:128] + ps_odd[0, 128:256]; one PSUM
                # operand per vector op, so stage one half through ACT
                hb = small.tile([1, H], F32, tag="hb")
                nc.scalar.copy(hb, ps_odd[0:1, H : 2 * H])

                recip = small.tile([1, 1], F32, tag="recip")
                nc.vector.reciprocal(out=recip, in_=tot_ps)

                sum_row = small.tile([1, H], F32, tag="sum_row")
                nc.vector.tensor_add(sum_row, ps_even[0:1, 0:H], hb)
                # normalize on ACT (DVE tensor_scalar w/ AP scalar is slow)
                out_row = small.tile([1, H], F32, tag="out_row")
                nc.scalar.activation(
                    out=out_row,
                    in_=sum_row,
                    func=mybir.ActivationFunctionType.Copy,
                    scale=recip,
                )
                # Issue on the scalar engine's HWDGE ring: on the sync ring
                # this DMA's wait (on out_row) would stall the SP sequencer
                # and block the next batches' x-load DMAs queued behind it.
                nc.scalar.dma_start(out=out_ext[b : b + 1, :], in_=out_row)

            for b in range(B_SHARD):
                for stage in (stage0, stage1, stage2, stage3, stage4):
                    stage(b, st[b])
                if b > 1:
                    stage5(b - 2, st[b - 2])
            for tail in (2, 1):
                if B_SHARD - tail >= 0:
                    stage5(B_SHARD - tail, st[B_SHARD - tail])

    # Bacc pipeline: splits multi-sem waits (HW allows one per instr),
    # inserts GPSIMD library loads + ACT table loads, lowers extended ISA.
    nc.compile()
    return nc


def _get_nc() -> bass.Bass:
    global _nc_cache
    if _nc_cache is None:
        _nc_cache = _build()
    return _nc_cache


def run(encoder_outputs: np.ndarray, attention_weights: np.ndarray, **spmd_kwargs):
    """Run the SPMD kernel; returns (output [B, H], BassKernelResults)."""
    nc = _get_nc()
    x = np.ascontiguousarray(encoder_outputs, dtype=np.float32)
    w = np.ascontiguousarray(attention_weights, dtype=np.float32)
    assert x.shape == (B, S, H), x.shape
    assert w.shape == (H, 1), w.shape
    in_maps = [
        {
            "encoder_outputs": x[i * B_SHARD : (i + 1) * B_SHARD],
            "attention_weights": w,
        }
        for i in range(N_CORES)
    ]
    res = run_bass_kernel_spmd(nc, in_maps, core_ids=list(range(N_CORES)), **spmd_kwargs)
    out = np.concatenate(
        [res.results[i]["out"] for i in range(N_CORES)], axis=0
    ).astype(np.float32)
    return out, res


def kernel(encoder_outputs: np.ndarray, attention_weights: np.ndarray) -> np.ndarray:
    out, _ = run(encoder_outputs, attention_weights)
    return out


# revision 67
# speedup vs baseline: 1.0711x; 1.0711x over previous
"""Attention-pooling kernel for Trainium2 (8 NeuronCores, SPMD data-parallel).

Computes, for x: [B, S, H] and w: [H, 1]:
    scores[b, s] = sum_h tanh(x[b, s, h]) * w[h]
    attn = softmax(scores, axis=s)
    out[b, h]   = sum_s attn[b, s] * x[b, s, h]

Sharding: data-parallel over batch B across 8 cores (32 batches/core),
w replicated. No inter-core communication; host concatenates the shards.

Per-core dataflow (per batch b), s-tile t in [0, 32), s = p*32 + t:
  DMA   : x[b] -> SBUF as [128 part, 32 tile, 128 h]  (16 KB contiguous
          per partition; float32r-typed view of the same bytes)
  ACT   : energy = tanh(x), split into a GPSIMD range [0, GS) and a DVE
          range [GS, 32) so each multiply has one owner engine
  GPSIMD: eg *= w  (in place);  DVE: ev *= w  (in place)
  DVE   : scores = reduce_add(energy, axis=h)          [128, 32]
  ACT   : ebuf = exp(scores) (float32r), accum_out -> rowsum [128, 1]
  PE    : context via fp32r M=1 matmuls over tile pairs (the fp32r fast
          path needs a moving free size >= 256): even tiles accumulate
          into ps_even[0, 0:128], odd tiles into ps_odd[0, 128:256];
          the unused half of each stream is discarded.  Both useful
          halves land on partition 0, so engines can read them.
  PE    : total = rowsum.T @ ones    [1, 1]
  ACT   : hb = copy(ps_odd[0, 128:256])   (ACT sits close to PSUM)
  DVE   : sum_row = ps_even[0, 0:128] + hb;  recip = 1/total
  ACT   : out_row = sum_row * recip  (scale-AP copy); DMA 512 B -> out[b]

The epilogue of batch b is deferred two batches (emitted after batch
b+2's pair-matmuls) so ACT's in-order stream doesn't stall the front
of later chains on PE: distance 2 measured -18 us vs distance 1.
Softmax normalization is algebraically factored out of the weighted sum
(exp without max-subtraction is safe: |scores| < ~40 here).
"""

import numpy as np

import concourse.bass as bass
import concourse.tile as tile
from concourse import bacc, mybir
from concourse.bass_utils import run_bass_kernel_spmd

B, S, H = 256, 4096, 128
N_CORES = 8
B_SHARD = B // N_CORES  # 32
P = 128                 # SBUF partitions; also H
S_TILES = S // P        # 32  (s = p * S_TILES + t)

F32 = mybir.dt.float32
F32R = mybir.dt.float32r

# s-tiles [0, GS) of the score multiply run on GPSIMD, [GS, S_TILES) on
# DVE. HW-swept: 16 is a sharp optimum (14 -> 276, 21 -> 296, 22 -> 356):
# lighter GPSIMD SBUF traffic decongests DVE/DMA more than the extra DVE
# multiply tiles cost.
GS = 16

_nc_cache = None


def _build() -> bass.Bass:
    nc = bacc.Bacc(None, target_bir_lowering=False, enable_partition_id=False)

    x_ext = nc.declare_dram_parameter(
        "encoder_outputs", [B_SHARD, S, H], F32, isOutput=False
    )
    w_ext = nc.declare_dram_parameter(
        "attention_weights", [H, 1], F32, isOutput=False
    )
    out_ext = nc.declare_dram_parameter("out", [B_SHARD, H], F32, isOutput=True)

    gs = max(1, min(GS, S_TILES - 1))
    vs = S_TILES - gs

    with tile.TileContext(nc) as tc:
        with (
            tc.tile_pool(name="singles", bufs=1) as singles,
            tc.tile_pool(name="xpool", bufs=6) as xpool,
            tc.tile_pool(name="egpool", bufs=4) as egpool,
            tc.tile_pool(name="evpool", bufs=4) as evpool,
            tc.tile_pool(name="small", bufs=8) as small,
            tc.tile_pool(name="psum_ctx", bufs=3, space="PSUM") as psum_ctx_pool,
            tc.tile_pool(name="psum_tot", bufs=2, space="PSUM") as psum_tot_pool,
        ):
            # w broadcast across partitions: w_bcast[p, h] = w[h]
            w_bcast = singles.tile([P, H], F32)
            w_flat = w_ext[:].rearrange("h one -> (one h)")
            w_row = bass.AP(
                tensor=w_flat.tensor,
                offset=w_flat.offset,
                ap=[[0, P], w_flat.ap[0]],
            )
            nc.sync.dma_start(out=w_bcast, in_=w_row)

            ones_col = singles.tile([P, 1], F32)
            nc.vector.memset(ones_col, 1.0)

            # w replicated along the tile axis, one private copy per
            # consumer engine (concurrent same-address reads from two
            # engines contend on SBUF ports)
            w_rep_g = singles.tile([P, gs, H], F32)
            for t in range(gs):
                nc.vector.tensor_copy(w_rep_g[:, t, :], w_bcast)
            w_rep_v = singles.tile([P, vs, H], F32)
            for t in range(vs):
                nc.vector.tensor_copy(w_rep_v[:, t, :], w_bcast)

            # [b, p, t, h] view of DRAM; partition p reads 16 KB contiguous
            xv = x_ext[:].rearrange("b (p t) h -> b p t h", p=P)

            # Per-batch stages; state is carried in dicts because batch
            # b's epilogue (stage5) is emitted two batches late.
            st = [dict() for _ in range(B_SHARD)]

            def stage0(b, d):  # load
                d["xb"] = xb = xpool.tile([P, S_TILES, H], F32R, tag="xb", name="xb")
                nc.sync.dma_start(out=xb, in_=xv[b].bitcast(F32R))

            def stage1(b, d):  # tanh
                xbf = d["xb"].bitcast(F32)
                d["eg"] = eg = egpool.tile([P, gs, H], F32, tag="eg", name="eg")
                d["ev"] = ev = evpool.tile([P, vs, H], F32, tag="ev", name="ev")
                nc.scalar.activation(
                    out=eg,
                    in_=xbf[:, 0:gs, :],
                    func=mybir.ActivationFunctionType.Tanh,
                )
                nc.scalar.activation(
                    out=ev,
                    in_=xbf[:, gs:, :],
                    func=mybir.ActivationFunctionType.Tanh,
                )

            def stage2(b, d):  # score multiply (split GPSIMD / DVE),
                # both in place: out-of-place DVE TT measured ~25% slower
                eg, ev = d["eg"], d["ev"]
                nc.vector.tensor_mul(ev, ev, w_rep_v)
                nc.gpsimd.tensor_mul(eg, eg, w_rep_g)

            def stage3(b, d):  # reduce + exp (each engine reduces its
                # own range, so the two score chains stay decoupled)
                scores = small.tile([P, S_TILES], F32, tag="scores")
                nc.vector.tensor_reduce(
                    out=scores[:, gs:],
                    in_=d["ev"],
                    axis=mybir.AxisListType.X,
                    op=mybir.AluOpType.add,
                )
                nc.vector.tensor_reduce(
                    out=scores[:, 0:gs],
                    in_=d["eg"],
                    axis=mybir.AxisListType.X,
                    op=mybir.AluOpType.add,
                )
                d["ebuf"] = ebuf = small.tile([P, S_TILES], F32R, tag="ebuf", name="ebuf")
                d["rowsum"] = rowsum = small.tile([P, 1], F32, tag="rowsum", name="rowsum")
                nc.scalar.activation(
                    out=ebuf,
                    in_=scores,
                    func=mybir.ActivationFunctionType.Exp,
                    accum_out=rowsum,
                )

            def stage4(b, d):  # fp32r pair-matmuls
                xb, ebuf = d["xb"], d["ebuf"]
                ps_even = psum_ctx_pool.tile([1, 2 * H], F32, tag="ps_even")
                ps_odd = psum_ctx_pool.tile([1, 2 * H], F32, tag="ps_odd")
                npairs = S_TILES // 2
                for j in range(npairs):
                    rhs = xb[:, 2 * j : 2 * j + 2, :]
                    nc.tensor.matmul(
                        ps_even,
                        ebuf[:, 2 * j : 2 * j + 1],
                        rhs,
                        start=(j == 0),
                        stop=(j == npairs - 1),
                    )
                    nc.tensor.matmul(
                        ps_odd,
                        ebuf[:, 2 * j + 1 : 2 * j + 2],
                        rhs,
                        start=(j == 0),
                        stop=(j == npairs - 1),
                    )

                tot_ps = psum_tot_pool.tile([1, 1], F32)
                nc.tensor.matmul(
                    tot_ps, d["rowsum"], ones_col, start=True, stop=True
                )
                d["ps_even"], d["ps_odd"], d["tot_ps"] = ps_even, ps_odd, tot_ps

            def stage5(b, d):  # normalize + store (emitted one batch late
                # so ACT's PSUM-gated epilogue ops don't sit between the
                # next batch's tanh ops in ACT's in-order stream)
                ps_even, ps_odd, tot_ps = d["ps_even"], d["ps_odd"], d["tot_ps"]
                # ctx = ps_even[0, 0:128] + ps_odd[0, 128:256]; one PSUM
                # operand per vector op, so stage one half through ACT
                hb = small.tile([1, H], F32, tag="hb")
                nc.scalar.copy(hb, ps_odd[0:1, H : 2 * H])

                recip = small.tile([1, 1], F32, tag="recip")
                nc.vector.reciprocal(out=recip, in_=tot_ps)

                sum_row = small.tile([1, H], F32, tag="sum_row")
                nc.vector.tensor_add(sum_row, ps_even[0:1, 0:H], hb)
                # normalize on ACT (DVE tensor_scalar w/ AP scalar is slow)
                out_row = small.tile([1, H], F32, tag="out_row")
                nc.scalar.activation(
                    out=out_row,
                    in_=sum_row,
                    func=mybir.ActivationFunctionType.Copy,
                    scale=recip,
                )
                # Issue on the scalar engine's HWDGE ring: on the sync ring
                # this DMA's wait (on out_row) would stall the SP sequencer
                # and block the next batches' x-load DMAs queued behind it.
                nc.scalar.dma_start(out=out_ext[b : b + 1, :], in_=out_row)

            for b in range(B_SHARD):
                for stage in (stage0, stage1, stage2, stage3, stage4):
                    stage(b, st[b])
                if b > 1:
                    stage5(b - 2, st[b - 2])
            for tail in (2, 1):
                if B_SHARD - tail >= 0:
                    stage5(B_SHARD - tail, st[B_SHARD - tail])

    # Bacc pipeline: splits multi-sem waits (HW allows one per instr),
    # inserts GPSIMD library loads + ACT table loads, lowers extended ISA.
    nc.compile()
    return nc


def _get_nc() -> bass.Bass:
    global _nc_cache
    if _nc_cache is None:
        _nc_cache = _build()
    return _nc_cache


def run(encoder_outputs: np.ndarray, attention_weights: np.ndarray, **spmd_kwargs):
    """Run the SPMD kernel; returns (output [B, H], BassKernelResults)."""
    nc = _get_nc()
    x = np.ascontiguousarray(encoder_outputs, dtype=np.float32)
    w = np.ascontiguousarray(attention_weights, dtype=np.float32)
    assert x.shape == (B, S, H), x.shape
    assert w.shape == (H, 1), w.shape
    in_maps = [
        {
            "encoder_outputs": x[i * B_SHARD : (i + 1) * B_SHARD],
            "attention_weights": w,
        }
        for i in range(N_CORES)
    ]
    res = run_bass_kernel_spmd(nc, in_maps, core_ids=list(range(N_CORES)), **spmd_kwargs)
    out = np.concatenate(
        [res.results[i]["out"] for i in range(N_CORES)], axis=0
    ).astype(np.float32)
    return out, res


def kernel(encoder_outputs: np.ndarray, attention_weights: np.ndarray) -> np.ndarray:
    out, _ = run(encoder_outputs, attention_weights)
    return out
